# revision 9
# baseline (speedup 1.0000x reference)
"""Bahdanau additive-attention kernel for Trainium2, 8 NeuronCores.

Problem (B=32, S=2048, H=1024, E=2H):
    hid_proj = hidden @ w_h.T + attn_b                  # (B, H)
    enc_proj[b,s,h] = sum_e enc[b,s,e] * w_e[h,e]       # (B, S, H)   <- dominant
    energy = tanh(hid_proj[:,None,:] + enc_proj)
    scores[b,s] = sum_h energy[b,s,h] * v_w[h]
    attw = softmax(scores, axis=1)
    context[b,e] = sum_s attw[b,s] * enc[b,s,e]

Sharding: data-parallel over batch, 4 batches per core.

Per-core dataflow (one Tile graph, pipelined; DMA instruction count kept low
because each DMA costs ~625ns of serialized HWDGE issue time):
  phase 1, per s-512 tile: 4x DMA fp32 (128,2048) -> DVE cast bf16 ->
  one batched DMA to a bf16 DRAM scratch (for phase 2) + ONE xbar DMA
  transpose (3D out) producing encT (e-part, s-free); PE matmul vs resident
  w_eT bf16 (K=2048 PSUM-accumulated); ACT tanh with fused per-partition
  bias; PE v-dot -> scores row.
  softmax tail, per batch: ACT exp with fused accum (Z), DVE reciprocal; PE
  transposes the UNnormalized exp row into per-s-block bf16 columns; context
  matmul accumulates over s in PSUM vs bf16 scratch tiles streamed back; the
  1/Z normalization is folded into the PSUM drains (context) and an
  off-critical-path ACT scale (attention-weight output row). No max
  subtraction in softmax: |scores| ~ 1 by construction.
"""

import numpy as np
import ml_dtypes

import concourse.bass as bass
import concourse.tile as tile
import concourse.mybir as mybir
from concourse import bacc
from concourse.bass_utils import run_bass_kernel_spmd

B, S, H = 32, 2048, 1024
E = 2 * H
NCORES = 8
BL = B // NCORES          # batches per core
P = 128                   # partitions
KC = E // P               # 16 contraction chunks
MC = H // P               # 8 h chunks
NT = 512                  # moving free-dim per matmul (1 PSUM bank of fp32)
ST = S // NT              # 4 s-tiles per batch
SJ = S // P               # 16 s-128 blocks per batch
QT = NT // P              # 4 s-128 blocks per s-tile

F32 = mybir.dt.float32
BF16 = mybir.dt.bfloat16
BF16_NP = ml_dtypes.bfloat16


def build_nc(reps=1):
    nc = bacc.Bacc("TRN2", target_bir_lowering=False, debug=False)

    enc_in = nc.dram_tensor("enc", [BL, S, E], F32, kind="ExternalInput")
    w_in = nc.dram_tensor("wT", [KC, P, H], BF16, kind="ExternalInput")
    v_in = nc.dram_tensor("vcol", [P, MC], BF16, kind="ExternalInput")
    hid_in = nc.dram_tensor("hidc", [P, BL, MC], F32, kind="ExternalInput")
    ctx_out = nc.dram_tensor("ctx", [BL, E], F32, kind="ExternalOutput")
    attw_out = nc.dram_tensor("attw", [BL, S], F32, kind="ExternalOutput")

    with tile.TileContext(nc) as tc:
        with (
            tc.tile_pool(name="singles", bufs=1) as singles,
            tc.tile_pool(name="pf32", bufs=2) as pf32,
            tc.tile_pool(name="pb16", bufs=2) as pb16,
            tc.tile_pool(name="pT", bufs=2) as pT,
            tc.tile_pool(name="pen", bufs=2) as pen,
            tc.tile_pool(name="pnt", bufs=2) as pnt,
            tc.tile_pool(name="prow", bufs=1) as prow,
            tc.tile_pool(name="pscore", bufs=2) as pscore,
            tc.tile_pool(name="pwcol", bufs=2) as pwcol,
            tc.tile_pool(name="pmm", bufs=2, space="PSUM") as pmm,
            tc.tile_pool(name="psc", bufs=1, space="PSUM") as psc,
            tc.tile_pool(name="pwc", bufs=1, space="PSUM") as pwc,
            tc.tile_pool(name="pctx", bufs=1, space="PSUM") as pctx,
            tc.tile_pool(name="pdram", bufs=1, space="DRAM") as pdram,
        ):
            # resident weights
            w_sb = singles.tile([P, KC, H], BF16)
            for k in range(KC):
                nc.sync.dma_start(out=w_sb[:, k, :], in_=w_in[k])
            v_sb = singles.tile([P, MC], BF16)
            nc.sync.dma_start(out=v_sb, in_=v_in[:, :])
            hid_sb = singles.tile([P, BL, MC], F32)
            nc.sync.dma_start(out=hid_sb, in_=hid_in[:, :, :])
            ident = singles.tile([1, 1], F32)
            nc.vector.memset(ident, 1.0)

            # bf16 copy of the encoder rows, natural layout (phase-2 input)
            encb = pdram.tile([BL, ST, P, QT, E], BF16)

            for rep_b in range(reps * BL):
                rep, b = divmod(rep_b, BL)
                scores_row = pscore.tile([1, S], F32)
                for st in range(ST):
                    b16 = pb16.tile([P, QT, E], BF16)
                    for q in range(QT):
                        j = st * QT + q
                        f32 = pf32.tile([P, E], F32)
                        nc.sync.dma_start(
                            out=f32, in_=enc_in[b, j * P : (j + 1) * P, :]
                        )
                        nc.vector.tensor_copy(out=b16[:, q, :], in_=f32)
                    nc.sync.dma_start(out=encb[b, st], in_=b16)
                    # one xbar transpose for the whole s-512 tile:
                    # encT[p, q, k, s] = b16[s, q, k*128+p]
                    encT = pT.tile([P, QT, KC, P], BF16)
                    nc.sync.dma_start_transpose(out=encT, in_=b16)
                    en = pen.tile([P, MC, NT], BF16)
                    for m in range(MC):
                        ps = pmm.tile([P, NT], F32)
                        for k in range(KC):
                            nc.tensor.matmul(
                                ps,
                                lhsT=w_sb[:, k, m * P : (m + 1) * P],
                                rhs=encT[:, :, k, :],
                                start=(k == 0),
                                stop=(k == KC - 1),
                            )
                        nc.scalar.activation(
                            out=en[:, m, :],
                            in_=ps,
                            func=mybir.ActivationFunctionType.Tanh,
                            bias=hid_sb[:, b, m : m + 1],
                            scale=1.0,
                        )
                    sc = psc.tile([1, NT], F32)
                    for m in range(MC):
                        nc.tensor.matmul(
                            sc,
                            lhsT=v_sb[:, m : m + 1],
                            rhs=en[:, m, :],
                            start=(m == 0),
                            stop=(m == MC - 1),
                        )
                    nc.vector.tensor_copy(
                        out=scores_row[:, st * NT : (st + 1) * NT], in_=sc
                    )

                # softmax: exp with fused free-dim accumulation -> Z
                ex = prow.tile([1, S], F32)
                zt = prow.tile([1, 1], F32)
                nc.scalar.activation(
                    out=ex,
                    in_=scores_row,
                    func=mybir.ActivationFunctionType.Exp,
                    accum_out=zt,
                )
                rz = prow.tile([1, 1], F32)
                nc.vector.reciprocal(out=rz, in_=zt)

                # attention-weight output row (off the context critical path)
                wrow = prow.tile([1, S], F32)
                nc.scalar.activation(
                    out=wrow,
                    in_=ex,
                    func=mybir.ActivationFunctionType.Identity,
                    scale=rz,
                )
                nc.sync.dma_start(out=attw_out[b], in_=wrow)

                # transpose UNnormalized exp row into per-s-block bf16 columns
                wcols = pwcol.tile([P, SJ], BF16)
                for t in range(SJ):
                    pw = pwc.tile([P, 1], F32)
                    nc.tensor.transpose(pw, ex[:, t * P : (t + 1) * P], ident)
                    nc.vector.tensor_copy(out=wcols[:, t : t + 1], in_=pw)

                # context: ctx[e] = (sum_s exp[s] * enc[s, e]) / Z
                cps = [
                    pctx.tile([1, NT], F32, name=f"cps{g}_{b}_{rep}", tag=f"cps{g}")
                    for g in range(4)
                ]
                for st in range(ST):
                    nt = pnt.tile([P, QT, E], BF16)
                    nc.sync.dma_start(out=nt, in_=encb[b, st])
                    for q in range(QT):
                        j = st * QT + q
                        for g in range(4):
                            nc.tensor.matmul(
                                cps[g],
                                lhsT=wcols[:, j : j + 1],
                                rhs=nt[:, q, g * NT : (g + 1) * NT],
                                start=(j == 0),
                                stop=(j == SJ - 1),
                            )
                ctxrow = prow.tile([1, E], F32)
                for g in range(4):
                    nc.vector.tensor_scalar_mul(
                        ctxrow[:, g * NT : (g + 1) * NT], cps[g], rz
                    )
                nc.sync.dma_start(out=ctx_out[b], in_=ctxrow)

    nc.compile()
    return nc


_CACHE = {}


def _get_nc():
    if "nc" not in _CACHE:
        _CACHE["nc"] = build_nc()
    return _CACHE["nc"]


def prep_in_maps(hidden, encoder_outputs, attn_w, attn_b, v_w):
    hidden = np.asarray(hidden, dtype=np.float32)
    enc = np.asarray(encoder_outputs, dtype=np.float32)
    attn_w = np.asarray(attn_w, dtype=np.float32)
    attn_b = np.asarray(attn_b, dtype=np.float32)
    v_w = np.asarray(v_w, dtype=np.float32)

    # host-side prep of the small operands
    w_h = attn_w[:, :H]                       # (H, H)
    w_e = attn_w[:, H:]                       # (H, E)
    hid_proj = hidden @ w_h.T + attn_b        # (B, H) fp32, exact
    wT = np.ascontiguousarray(w_e.T).astype(BF16_NP).reshape(KC, P, H)
    vcol = np.ascontiguousarray(v_w.reshape(MC, P).T).astype(BF16_NP)  # (P, MC)

    in_maps = []
    for c in range(NCORES):
        hp = hid_proj[c * BL : (c + 1) * BL]  # (BL, H)
        # hidc[p, b, m] = hid_proj[b, m*128+p]
        hidc = np.ascontiguousarray(hp.reshape(BL, MC, P).transpose(2, 0, 1))
        in_maps.append(
            {
                "enc": enc[c * BL : (c + 1) * BL],
                "wT": wT,
                "vcol": vcol,
                "hidc": hidc.astype(np.float32),
            }
        )
    return in_maps


def kernel(hidden, encoder_outputs, attn_w, attn_b, v_w):
    in_maps = prep_in_maps(hidden, encoder_outputs, attn_w, attn_b, v_w)
    nc = _get_nc()
    res = run_bass_kernel_spmd(nc, in_maps, core_ids=list(range(NCORES)))
    ctx = np.concatenate([res.results[c]["ctx"] for c in range(NCORES)], axis=0)
    attw = np.concatenate([res.results[c]["attw"] for c in range(NCORES)], axis=0)
    return ctx.astype(np.float32), attw.astype(np.float32)
